# revision 44
# baseline (speedup 1.0000x reference)
"""Distributed Trainium2 Bass kernel for nn_CausalSelfAttention_66984309948568.

Strategy (8 NeuronCores, tensor-parallel over heads):
  - core h owns head h (8 heads, head_dim 128).
  - host pre-transposes x -> x^T (bf16, to halve the dominant DMA stream)
    and slices per-head weights (bf16); attention operands are float32r
    (FP22 read path, full PE rate at N=512).
  - per core: qkv projection in [d, t] layout; RMS-norm factors via
    ones-matmul column sums + exp(-0.5*ln(mean+eps)); factors broadcast
    across partitions on GpSimd; RoPE applied in place (64-partition
    rotated copy built on GpSimd, full-width mult/adds on DVE) before the
    norm multiply; causal attention per k-tile in S^T = [s, t] layout with
    4-deep single-bank score PSUM buffering; softmax denominators from
    accumulating ones-column matmuls; y normalized via GpSimd-broadcast
    reciprocal; AllToAll exchanges per-head y slices so each core projects
    its own 512 rows of output with per-head streamed inputs accumulating
    into 8 PSUM banks.
  - all scalar-engine activations are forced onto the combined
    natural_log_exp table set (one load instead of 30+ reloads).
  - host concatenates the 8 [512, 1024] slices.
"""

import sys

sys.path.insert(0, "/opt/trn_rl_repo")

import numpy as np
import concourse.bass as bass
import concourse.bacc as bacc
import concourse.bass_isa as bass_isa
import concourse.mybir as mybir
from concourse import tile
from concourse.bass_utils import run_bass_kernel_spmd
from concourse.hw_specs import get_activation_tables

N_CORES = 8
B, T, DIM = 1, 4096, 1024
NUM_HEADS, HEAD_DIM = 8, 128
HDIM = NUM_HEADS * HEAD_DIM
SCALE = 0.12
EPS = 1.1920928955078125e-07
NCHUNK = T // 512          # 8 t-chunks of 512
NTT = T // 128             # 32 t-tiles of 128
TSLICE = T // N_CORES      # 512 output rows per core

f32 = mybir.dt.float32
f32r = mybir.dt.float32r
bf16 = mybir.dt.bfloat16
FN = mybir.ActivationFunctionType
ALU = mybir.AluOpType
MASK_NEG = -30000.0


def _trunc22(a):
    b = np.ascontiguousarray(a, dtype=np.float32).copy()
    b.view(np.uint32)[...] &= 0xFFFFFC00
    return b


def _register_const(nc, value, dtype=f32):
    if (dtype, value) in nc.const_aps.aps:
        return
    t = nc.alloc_sbuf_tensor(f"const-{dtype.name}-{value}", [128, 1], dtype)
    nc.gpsimd.memset(t.ap(), value)
    nc.const_aps.aps[(dtype, value)] = t.ap()


def _force_single_act_table(nc):
    """Re-target every LoadActFuncSet to the combined natural_log_exp set and
    drop the now-redundant repeats (the set never changes after the first
    load on each control-flow path)."""
    tables = list(get_activation_tables(nc.m.arch).keys())
    combined = tables.index("natural_log_exp_and_others")
    need = {FN.Ln, FN.Exp, FN.Copy, FN.Square, FN.Identity}
    assert need <= get_activation_tables(nc.m.arch)["natural_log_exp_and_others"]
    for fn in nc.m.functions:
        for blk in fn.blocks:
            first_seen = False
            keep = []
            for inst in blk.instructions:
                if isinstance(inst, mybir.InstLoadActFuncSet):
                    assert inst.sync_info is None or (
                        not inst.sync_info.on_wait and not inst.sync_info.on_update
                    )
                    if first_seen:
                        continue  # drop duplicate load
                    inst.act_func_set_id = combined
                    first_seen = True
                keep.append(inst)
            blk.instructions[:] = keep


PHASE_MARKS = {}


def _build_program(repeat=1):
    nc = bacc.Bacc(num_devices=N_CORES)
    _register_const(nc, EPS)
    _register_const(nc, float(np.log(SCALE)))
    nc.all_engine_barrier()

    # ---- DRAM parameters (per-core values supplied via in_maps) ----
    xt_d = nc.declare_dram_parameter("xt", [DIM, T], bf16, isOutput=False)
    wqkv_d = nc.declare_dram_parameter("wqkv", [128, 3 * DIM], bf16, isOutput=False)
    vew_d = nc.declare_dram_parameter("vew", [128, T], f32, isOutput=False)
    cos_d = nc.declare_dram_parameter("cos1", [64, T], f32, isOutput=False)
    sin_d = nc.declare_dram_parameter("sin0", [64, T], f32, isOutput=False)
    mask_d = nc.declare_dram_parameter("maskc", [128, 2048], f32, isOutput=False)
    pw_d = nc.declare_dram_parameter("pw", [128, 8 * DIM], f32r, isOutput=False)
    onc_d = nc.declare_dram_parameter("ones_col", [128, 1], f32r, isOutput=False)
    id_d = nc.declare_dram_parameter("ident", [128, 128], f32, isOutput=False)
    out_d = nc.declare_dram_parameter("out", [TSLICE, DIM], f32, isOutput=True)

    ln_scale_q = float(np.log(SCALE))

    with tile.TileContext(nc, num_cores=N_CORES) as tc:
        with (
            tc.tile_pool(name="persist", bufs=1) as persist,
            tc.tile_pool(name="dram", bufs=1, space="DRAM") as dram,
        ):
            # persistent SBUF tensors
            qnT = persist.tile([128, T], f32r, tag="qnT")     # 0.12 * rope(norm(q))^T
            knT = persist.tile([128, T], f32r, tag="knT")     # rope(norm(k))^T
            v_sb = persist.tile([128, NTT * 128], f32r, tag="v_sb")  # v in [t,d] tiles
            maskc = persist.tile([128, 2048], f32, tag="maskc")
            onc = persist.tile([128, 1], f32r, tag="onc")
            ident = persist.tile([128, 128], f32, tag="ident")
            cmat = persist.tile([128, T], f32, tag="cmat")
            smat = persist.tile([128, T], f32, tag="smat")

            nc.gpsimd.dma_start(onc[:], onc_d[:])
            nc.gpsimd.dma_start(ident[:], id_d[:])
            def load_rope_tables():
                # cmat = [cos;1;cos;1], smat (pre-shifted) = [sin;0;-sin;0]:
                # DMA the [64,T] halves twice, negate rows 64:96 on DVE.
                nc.scalar.dma_start(cmat[0:64, :], cos_d[:])
                nc.scalar.dma_start(cmat[64:128, :], cos_d[:])
                nc.scalar.dma_start(smat[0:64, :], sin_d[:])
                nc.scalar.dma_start(smat[64:128, :], sin_d[:])
                nc.vector.tensor_scalar_mul(smat[64:96, :], smat[64:96, :], -1.0)
                nc.scalar.dma_start(maskc[:], mask_d[:])

            a2a_in = dram.tile([N_CORES * 128, TSLICE], f32r, tag="a2a_in")
            a2a_out = dram.tile([N_CORES * 128, TSLICE], f32r, tag="a2a_out")

            for _rep in range(repeat):
              PHASE_MARKS["p1"] = nc.next_id()
              # ================ Phase 1: qkv, norm, rope, v assembly ==========
              with (
                  tc.tile_pool(name="wpool", bufs=1) as wpool,
                  tc.tile_pool(name="xt", bufs=2) as xt_pool,
                  tc.tile_pool(name="qkv_ps", bufs=2, space=bass.MemorySpace.PSUM) as qkv_ps,
                  tc.tile_pool(name="row_ps", bufs=1, space=bass.MemorySpace.PSUM) as row_ps,
                  tc.tile_pool(name="tr_ps", bufs=1, space=bass.MemorySpace.PSUM) as tr_ps,
                  tc.tile_pool(name="evac", bufs=2) as evac,
                  tc.tile_pool(name="rows", bufs=2) as rows,
                  tc.tile_pool(name="tmps", bufs=3) as tmps,
              ):
                  wqkv = wpool.tile([128, 3 * DIM], bf16, tag="wqkv")
                  nc.sync.dma_start(wqkv[:], wqkv_d[:])

                  for c in range(NCHUNK):
                      cs = bass.ts(c, 512)
                      ps_q = qkv_ps.tile([128, 512], f32, tag="ps_q")
                      ps_k = qkv_ps.tile([128, 512], f32, tag="ps_k")
                      ps_v = qkv_ps.tile([128, 512], f32, tag="ps_v")
                      # one 2MB DMA per chunk: [p, dt, col] <- xt[128*dt + p, 512c + col]
                      xt_t = xt_pool.tile([128, 8, 512], bf16, tag="xt")
                      nc.sync.dma_start(
                          xt_t[:],
                          xt_d[:, 512 * c : 512 * (c + 1)].rearrange(
                              "(dt p) col -> p dt col", p=128
                          ),
                      )
                      vew_c = xt_pool.tile([128, 512], f32, tag="vew_c")
                      nc.sync.dma_start(vew_c[:], vew_d[:, cs])
                      for dt in range(8):
                          st, sp = dt == 0, dt == 7
                          nc.tensor.matmul(ps_q[:], wqkv[:, bass.ts(dt, 128)], xt_t[:, dt, :], start=st, stop=sp)
                          nc.tensor.matmul(ps_k[:], wqkv[:, DIM + 128 * dt : DIM + 128 * (dt + 1)], xt_t[:, dt, :], start=st, stop=sp)
                          nc.tensor.matmul(ps_v[:], wqkv[:, 2 * DIM + 128 * dt : 2 * DIM + 128 * (dt + 1)], xt_t[:, dt, :], start=st, stop=sp)
                      if c == 0 and _rep == 0:
                          load_rope_tables()

                      # ---- v: transpose [d,t]->[t,d] per 128-tile, add ve ----
                      vTc = evac.tile([128, 512], f32, tag="vTc")
                      nc.scalar.copy(vTc[:], ps_v[:])
                      for j in range(4):
                          i = 4 * c + j
                          ps_t = tr_ps.tile([128, 128], f32, tag="ps_t")
                          nc.tensor.transpose(ps_t[:], vTc[:, bass.ts(j, 128)], ident[:])
                          nc.vector.tensor_tensor(
                              v_sb[:, bass.ts(i, 128)], ps_t[:], vew_c[:, bass.ts(j, 128)], ALU.add
                          )

                      # ---- q, k: evac -> (sq -> rowsum -> rsq -> bcast) | rope,
                      #      then in-place normalize ----
                      for which, ps_x, dstT in (("q", ps_q, qnT), ("k", ps_k, knT)):
                          dcs = dstT[:, cs]
                          nc.scalar.copy(dcs, ps_x[:])
                          sqc = tmps.tile([128, 512], f32r, tag="sqc")
                          nc.scalar.activation(sqc[:], ps_x[:], FN.Square)
                          ps_row = row_ps.tile([1, 512], f32, tag="ps_row")
                          nc.tensor.matmul(ps_row[:], onc[:], sqc[:], start=True, stop=True)
                          # rsq = exp(-0.5 * ln(mean + eps)) [* SCALE for q]
                          lnr = rows.tile([1, 512], f32, tag="lnr")
                          nc.scalar.activation(lnr[:], ps_row[:], FN.Ln, bias=EPS, scale=1.0 / HEAD_DIM)
                          rsq = rows.tile([1, 512], f32, tag="rsq")
                          nc.scalar.activation(
                              rsq[:], lnr[:], FN.Exp,
                              bias=(ln_scale_q if which == "q" else 0.0), scale=-0.5,
                          )
                          bc = tmps.tile([128, 512], f32, tag="bc")
                          nc.gpsimd.partition_broadcast(bc[:], rsq[:])
                          # rope in place on dstT: build the 64-partition-rotated
                          # copy on Pool, then full-width mult/add on DVE
                          # (smat rows are pre-shifted on the host; rows 32:64
                          # and 96:128 of smat are zero so the add is identity
                          # there)
                          dsh = tmps.tile([128, 512], f32, tag="dsh")
                          nc.gpsimd.tensor_copy(dsh[0:64, :], dcs[64:128, :].bitcast(f32))
                          nc.gpsimd.tensor_copy(dsh[64:128, :], dcs[0:64, :].bitcast(f32))
                          ut = tmps.tile([128, 512], f32, tag="ut")
                          nc.vector.tensor_tensor(ut[:], dsh[:], smat[:, cs], ALU.mult)
                          nc.vector.tensor_tensor(dcs, dcs.bitcast(f32), cmat[:, cs], ALU.mult)
                          nc.vector.tensor_tensor(dcs, dcs.bitcast(f32), ut[:], ALU.add)
                          # normalize (per-column factor, broadcast across partitions)
                          nc.vector.tensor_tensor(dcs, dcs.bitcast(f32), bc[:], ALU.mult)

              PHASE_MARKS["p3"] = nc.next_id()
              # ================= Phase 3: causal attention ======================
              with (
                  tc.tile_pool(name="s_ps", bufs=4, space=bass.MemorySpace.PSUM) as s_ps,
                  tc.tile_pool(name="y_ps", bufs=2, space=bass.MemorySpace.PSUM) as y_ps,
                  tc.tile_pool(name="r_ps", bufs=2, space=bass.MemorySpace.PSUM) as r_ps,
                  tc.tile_pool(name="pt", bufs=6) as pt_pool,
                  tc.tile_pool(name="att_sb", bufs=2) as att_sb,
              ):
                  for c in range(NCHUNK):
                      cs = bass.ts(c, 512)
                      n_s = 4 * (c + 1)
                      ps_y = y_ps.tile([128, 512], f32, tag="ps_y")
                      ps_r = r_ps.tile([1, 512], f32, tag="ps_r")
                      # diagonal tiles first (their mask lengthens the chain);
                      # the stop tile is then mask-free for a short chunk tail
                      order = list(range(4 * c, n_s)) + list(range(0, 4 * c))
                      for pos, i in enumerate(order):
                          st, sp = pos == 0, pos == n_s - 1
                          ps_S = s_ps.tile([128, 512], f32, tag="ps_S")
                          nc.tensor.matmul(
                              ps_S[:], knT[:, bass.ts(i, 128)], qnT[:, cs],
                              start=True, stop=True,
                          )
                          k_idx = i - 4 * c
                          if k_idx >= 0:
                              w = 128 * (k_idx + 1)
                              nc.vector.tensor_tensor(
                                  ps_S[:, 0:w],
                                  ps_S[:, 0:w],
                                  maskc[:, 512 * k_idx : 512 * k_idx + w],
                                  ALU.add,
                              )
                          pT = pt_pool.tile([128, 512], f32r, tag="pT")
                          nc.scalar.activation(pT[:], ps_S[:], FN.Exp)
                          nc.tensor.matmul(
                              ps_y[:], v_sb[:, bass.ts(i, 128)], pT[:],
                              start=st, stop=sp,
                          )
                          nc.tensor.matmul(
                              ps_r[:], onc[:], pT[:],
                              start=st, stop=sp,
                          )
                      # normalize y chunk by 1/rowsum and ship to a2a buffer
                      rrec = att_sb.tile([1, 512], f32, tag="rrec")
                      nc.vector.reciprocal(rrec[:], ps_r[:])
                      brs = att_sb.tile([128, 512], f32, tag="brs")
                      nc.gpsimd.partition_broadcast(brs[:], rrec[:])
                      yn = att_sb.tile([128, 512], f32r, tag="yn")
                      nc.vector.tensor_tensor(yn[:], ps_y[:], brs[:], ALU.mult)
                      nc.scalar.dma_start(a2a_in[128 * c : 128 * (c + 1), :], yn[:])

              PHASE_MARKS["cc"] = nc.next_id()
              nc.gpsimd.collective_compute(
                  "AllToAll",
                  ALU.bypass,
                  replica_groups=[list(range(N_CORES))],
                  ins=[a2a_in[:].opt()],
                  outs=[a2a_out[:].opt()],
              )

              PHASE_MARKS["p4"] = nc.next_id()
              # ================= Phase 4: output projection =====================
              # accumulate over heads into 8 PSUM banks; stream per-head inputs
              with (
                  tc.tile_pool(name="proj_in", bufs=3) as proj_in,
                  tc.tile_pool(name="o_ps", bufs=1, space=bass.MemorySpace.PSUM) as o_ps,
                  tc.tile_pool(name="outp", bufs=3) as outp,
              ):
                  ps_os = []
                  for j in range(8):
                      ps_os.append(
                          o_ps.tile([128, 512], f32, tag=f"ps_o{j}", name=f"ps_o{j}")
                      )
                  for hh in range(8):
                      yT_h = proj_in.tile([128, TSLICE], f32r, tag="yT_h")
                      nc.sync.dma_start(yT_h[:], a2a_out[128 * hh : 128 * (hh + 1), :])
                      pw_h = proj_in.tile([128, DIM], f32r, tag="pw_h")
                      nc.scalar.dma_start(pw_h[:], pw_d[:, bass.ts(hh, DIM)])
                      for m in range(4):
                          for dc in range(2):
                              nc.tensor.matmul(
                                  ps_os[2 * m + dc][:],
                                  yT_h[:, bass.ts(m, 128)],
                                  pw_h[:, bass.ts(dc, 512)],
                                  start=(hh == 0), stop=(hh == 7),
                              )
                  for m in range(4):
                      for dc in range(2):
                          ob = outp.tile([128, 512], f32, tag="ob")
                          if dc == 0:
                              nc.vector.tensor_copy(ob[:], ps_os[2 * m + dc][:])
                              nc.scalar.dma_start(
                                  out_d[128 * m : 128 * (m + 1), 0:512], ob[:]
                              )
                          else:
                              nc.scalar.copy(ob[:], ps_os[2 * m + dc][:])
                              nc.sync.dma_start(
                                  out_d[128 * m : 128 * (m + 1), 512:1024], ob[:]
                              )

    nc.finalize()
    _force_single_act_table(nc)
    return nc


_PROGRAM = None


def _get_program():
    global _PROGRAM
    if _PROGRAM is None:
        _PROGRAM = _build_program()
    return _PROGRAM


def _host_prep(x, ve, qkv_w, lambdas, proj_w):
    x = np.asarray(x, dtype=np.float32).reshape(T, DIM)
    ve = np.asarray(ve, dtype=np.float32).reshape(T, HDIM)
    qkv_w = np.asarray(qkv_w, dtype=np.float32)
    lam = np.asarray(lambdas, dtype=np.float32)
    proj_w = np.asarray(proj_w, dtype=np.float32)

    import ml_dtypes
    xt = np.ascontiguousarray(x.T).astype(ml_dtypes.bfloat16)    # [DIM, T]

    # rope tables
    nfreq = HEAD_DIM // 4
    ang = (1.0 / 1024.0) ** np.linspace(0.0, 1.0, nfreq, dtype=np.float32)
    theta = np.arange(T, dtype=np.float32)[:, None] * ang[None, :]     # [T, 32]
    cosT = np.cos(theta).T.astype(np.float32)              # [32, T]
    sinT = np.sin(theta).T.astype(np.float32)
    cos1 = np.concatenate([cosT, np.ones((32, T), np.float32)], axis=0)   # [64, T]
    sin0 = np.concatenate([sinT, np.zeros((32, T), np.float32)], axis=0)  # [64, T]

    # causal masks for the 4 diagonal sub-positions
    maskc = np.zeros((128, 2048), np.float32)
    tri = np.where(
        np.arange(128)[:, None] > np.arange(128)[None, :], MASK_NEG, 0.0
    ).astype(np.float32)
    for k in range(4):
        maskc[:, 512 * k : 512 * k + 128 * k] = MASK_NEG
        maskc[:, 512 * k + 128 * k : 512 * k + 128 * (k + 1)] = tri

    ones_col = _trunc22(np.ones((128, 1), np.float32))
    ident = np.eye(128, dtype=np.float32)

    in_maps = []
    for h in range(N_CORES):
        hs = slice(128 * h, 128 * (h + 1))
        # weight layout: w[p, dt*128 + m] = W[m, dt*128 + p]
        Wq = qkv_w[0, hs, :]                                # [128, DIM]
        Wk = qkv_w[1, hs, :]
        Wv = qkv_w[2, hs, :] * lam[0]
        def wlay(W):
            # [m, (dt p)] -> [p, (dt m)]
            a = W.reshape(128, 8, 128)                      # [m, dt, p]
            return _trunc22(np.ascontiguousarray(a.transpose(2, 1, 0).reshape(128, DIM)))
        # vew[p, i*128 + c] = lam1 * ve[i*128 + p, h*128 + c]
        veh = (ve[:, hs] * lam[1]).reshape(NTT, 128, 128)   # [i, p, c]
        vew = np.ascontiguousarray(veh.transpose(1, 0, 2).reshape(128, T))
        # pw[p, n*DIM + D] = proj_w[D, 128n + p]
        pwh = proj_w.T.reshape(8, 128, DIM)                 # [n, e_p, D]
        pw = _trunc22(np.ascontiguousarray(pwh.transpose(1, 0, 2).reshape(128, 8 * DIM)))
        in_maps.append(
            {
                "xt": xt,
                "wqkv": np.concatenate([wlay(Wq), wlay(Wk), wlay(Wv)], axis=1).astype(ml_dtypes.bfloat16),
                "vew": vew.astype(np.float32),
                "cos1": cos1,
                "sin0": sin0,
                "maskc": maskc,
                "pw": pw,
                "ones_col": ones_col,
                "ident": ident,
            }
        )
    return in_maps


def kernel(x, ve, qkv_w, lambdas, proj_w):
    in_maps = _host_prep(x, ve, qkv_w, lambdas, proj_w)
    nc = _get_program()
    res = run_bass_kernel_spmd(nc, in_maps, list(range(N_CORES)))
    out = np.concatenate([res.results[c]["out"] for c in range(N_CORES)], axis=0)
    return out.reshape(B, T, DIM).astype(np.float32)


# ---------------------------------------------------------------------------
# Timing support (test.py only): run the program with device-resident inputs
# so repeated executions measure device time, and difference two repeat
# factors to cancel dispatch overhead.
# ---------------------------------------------------------------------------

def make_runner(in_maps, repeat=1):
    import jax
    from jax.sharding import Mesh, PartitionSpec, NamedSharding
    from jax.experimental.shard_map import shard_map
    from concourse import bass2jax
    from concourse.bass2jax import _bass_exec_p, partition_id_tensor

    bass2jax.install_neuronx_cc_hook()
    nc = _build_program(repeat)

    in_names, out_names, out_avals, zero_outs = [], [], [], []
    partition_name = nc.partition_id_tensor.name if nc.partition_id_tensor else None
    for alloc in nc.m.functions[0].allocations:
        if not isinstance(alloc, mybir.MemoryLocationSet):
            continue
        name = alloc.memorylocations[0].name
        if alloc.kind == "ExternalInput":
            if name != partition_name:
                in_names.append(name)
        elif alloc.kind == "ExternalOutput":
            out_names.append(name)
            shape = tuple(alloc.tensor_shape)
            dtype = mybir.dt.np(alloc.dtype)
            out_avals.append(jax.core.ShapedArray(shape, dtype))
            zero_outs.append(np.zeros(shape, dtype))
    n_params = len(in_names)
    n_outs = len(out_avals)
    all_in_names = list(in_names) + out_names
    if partition_name is not None:
        all_in_names.append(partition_name)
    donate = tuple(range(n_params, n_params + n_outs))

    def _body(*args):
        operands = list(args)
        if partition_name is not None:
            operands.append(partition_id_tensor())
        outs = _bass_exec_p.bind(
            *operands,
            out_avals=tuple(out_avals),
            in_names=tuple(all_in_names),
            out_names=tuple(out_names),
            lowering_input_output_aliases=(),
            sim_require_finite=True,
            sim_require_nnan=True,
            nc=nc,
        )
        return tuple(outs)

    devices = jax.devices()[:N_CORES]
    mesh = Mesh(np.asarray(devices), ("core",))
    in_specs = (PartitionSpec("core"),) * (n_params + n_outs)
    out_specs = (PartitionSpec("core"),) * n_outs
    fn = jax.jit(
        shard_map(_body, mesh=mesh, in_specs=in_specs, out_specs=out_specs, check_rep=False),
        donate_argnums=donate,
        keep_unused=True,
    )
    sh = NamedSharding(mesh, PartitionSpec("core"))
    concat_in = [
        jax.device_put(
            np.concatenate([np.asarray(in_maps[c][nm]) for c in range(N_CORES)], axis=0), sh
        )
        for nm in in_names
    ]
    zero_glob = [np.zeros((N_CORES * z.shape[0], *z.shape[1:]), z.dtype) for z in zero_outs]

    def run_once():
        zs = [jax.device_put(z, sh) for z in zero_glob]
        outs = fn(*concat_in, *zs)
        for o in outs:
            o.block_until_ready()
        return outs

    return run_once


# revision 46
# speedup vs baseline: 3.2824x; 3.2824x over previous
"""Distributed Trainium2 Bass kernel for nn_CausalSelfAttention_66984309948568.

Strategy (8 NeuronCores, tensor-parallel over heads):
  - core h owns head h (8 heads, head_dim 128).
  - host pre-transposes x -> x^T (bf16, to halve the dominant DMA stream)
    and slices per-head weights (bf16); attention operands are float32r
    (FP22 read path, full PE rate at N=512).
  - per core: qkv projection in [d, t] layout; RMS-norm factors via
    ones-matmul column sums + exp(-0.5*ln(mean+eps)); factors broadcast
    across partitions on GpSimd; RoPE applied in place (64-partition
    rotated copy built on GpSimd, full-width mult/adds on DVE) before the
    norm multiply; causal attention per k-tile in S^T = [s, t] layout with
    4-deep single-bank score PSUM buffering; softmax denominators split:
    even k-tiles via accumulating ones-column matmuls on the PE, odd
    k-tiles summed elementwise on DVE and folded in with one cleanup
    matmul per chunk; y normalized via GpSimd-broadcast reciprocal; the
    AllToAll payload and projection inputs are bf16 (halves the exposed
    collective tail); each core projects its own 512 output rows with
    per-head streamed inputs accumulating into 8 PSUM banks.
  - all scalar-engine activations are forced onto the combined
    natural_log_exp table set (one load instead of 30+ reloads).
  - host concatenates the 8 [512, 1024] slices.
"""

import sys

sys.path.insert(0, "/opt/trn_rl_repo")

import numpy as np
import concourse.bass as bass
import concourse.bacc as bacc
import concourse.bass_isa as bass_isa
import concourse.mybir as mybir
from concourse import tile
from concourse.bass_utils import run_bass_kernel_spmd
from concourse.hw_specs import get_activation_tables

N_CORES = 8
B, T, DIM = 1, 4096, 1024
NUM_HEADS, HEAD_DIM = 8, 128
HDIM = NUM_HEADS * HEAD_DIM
SCALE = 0.12
EPS = 1.1920928955078125e-07
NCHUNK = T // 512          # 8 t-chunks of 512
NTT = T // 128             # 32 t-tiles of 128
TSLICE = T // N_CORES      # 512 output rows per core

f32 = mybir.dt.float32
f32r = mybir.dt.float32r
bf16 = mybir.dt.bfloat16
FN = mybir.ActivationFunctionType
ALU = mybir.AluOpType
MASK_NEG = -30000.0


def _trunc22(a):
    b = np.ascontiguousarray(a, dtype=np.float32).copy()
    b.view(np.uint32)[...] &= 0xFFFFFC00
    return b


def _register_const(nc, value, dtype=f32):
    if (dtype, value) in nc.const_aps.aps:
        return
    t = nc.alloc_sbuf_tensor(f"const-{dtype.name}-{value}", [128, 1], dtype)
    nc.gpsimd.memset(t.ap(), value)
    nc.const_aps.aps[(dtype, value)] = t.ap()


def _force_single_act_table(nc):
    """Re-target every LoadActFuncSet to the combined natural_log_exp set and
    drop the now-redundant repeats (the set never changes after the first
    load on each control-flow path)."""
    tables = list(get_activation_tables(nc.m.arch).keys())
    combined = tables.index("natural_log_exp_and_others")
    need = {FN.Ln, FN.Exp, FN.Copy, FN.Square, FN.Identity}
    assert need <= get_activation_tables(nc.m.arch)["natural_log_exp_and_others"]
    for fn in nc.m.functions:
        for blk in fn.blocks:
            first_seen = False
            keep = []
            for inst in blk.instructions:
                if isinstance(inst, mybir.InstLoadActFuncSet):
                    assert inst.sync_info is None or (
                        not inst.sync_info.on_wait and not inst.sync_info.on_update
                    )
                    if first_seen:
                        continue  # drop duplicate load
                    inst.act_func_set_id = combined
                    first_seen = True
                keep.append(inst)
            blk.instructions[:] = keep


PHASE_MARKS = {}


def _build_program(repeat=1):
    nc = bacc.Bacc(num_devices=N_CORES)
    _register_const(nc, EPS)
    _register_const(nc, float(np.log(SCALE)))
    nc.all_engine_barrier()

    # ---- DRAM parameters (per-core values supplied via in_maps) ----
    xt_d = nc.declare_dram_parameter("xt", [DIM, T], bf16, isOutput=False)
    wqkv_d = nc.declare_dram_parameter("wqkv", [128, 3 * DIM], bf16, isOutput=False)
    vew_d = nc.declare_dram_parameter("vew", [128, T], f32, isOutput=False)
    cos_d = nc.declare_dram_parameter("cos1", [64, T], f32, isOutput=False)
    sin_d = nc.declare_dram_parameter("sin0", [64, T], f32, isOutput=False)
    mask_d = nc.declare_dram_parameter("maskc", [128, 2048], f32, isOutput=False)
    pw_d = nc.declare_dram_parameter("pw", [128, 8 * DIM], bf16, isOutput=False)
    onc_d = nc.declare_dram_parameter("ones_col", [128, 1], f32r, isOutput=False)
    id_d = nc.declare_dram_parameter("ident", [128, 128], f32, isOutput=False)
    out_d = nc.declare_dram_parameter("out", [TSLICE, DIM], f32, isOutput=True)

    ln_scale_q = float(np.log(SCALE))

    with tile.TileContext(nc, num_cores=N_CORES) as tc:
        with (
            tc.tile_pool(name="persist", bufs=1) as persist,
            tc.tile_pool(name="dram", bufs=1, space="DRAM") as dram,
        ):
            # persistent SBUF tensors
            qnT = persist.tile([128, T], f32r, tag="qnT")     # 0.12 * rope(norm(q))^T
            knT = persist.tile([128, T], f32r, tag="knT")     # rope(norm(k))^T
            v_sb = persist.tile([128, NTT * 128], f32r, tag="v_sb")  # v in [t,d] tiles
            maskc = persist.tile([128, 2048], f32, tag="maskc")
            onc = persist.tile([128, 1], f32r, tag="onc")
            ident = persist.tile([128, 128], f32, tag="ident")
            cmat = persist.tile([128, T], f32, tag="cmat")
            smat = persist.tile([128, T], f32, tag="smat")

            nc.gpsimd.dma_start(onc[:], onc_d[:])
            nc.gpsimd.dma_start(ident[:], id_d[:])
            def load_rope_tables():
                # cmat = [cos;1;cos;1], smat (pre-shifted) = [sin;0;-sin;0]:
                # DMA the [64,T] halves twice, negate rows 64:96 on DVE.
                nc.scalar.dma_start(cmat[0:64, :], cos_d[:])
                nc.scalar.dma_start(cmat[64:128, :], cos_d[:])
                nc.scalar.dma_start(smat[0:64, :], sin_d[:])
                nc.scalar.dma_start(smat[64:128, :], sin_d[:])
                nc.vector.tensor_scalar_mul(smat[64:96, :], smat[64:96, :], -1.0)
                nc.scalar.dma_start(maskc[:], mask_d[:])

            a2a_in = dram.tile([N_CORES * 128, TSLICE], bf16, tag="a2a_in")
            a2a_out = dram.tile([N_CORES * 128, TSLICE], bf16, tag="a2a_out")

            for _rep in range(repeat):
              PHASE_MARKS["p1"] = nc.next_id()
              # ================ Phase 1: qkv, norm, rope, v assembly ==========
              with (
                  tc.tile_pool(name="wpool", bufs=1) as wpool,
                  tc.tile_pool(name="xt", bufs=2) as xt_pool,
                  tc.tile_pool(name="qkv_ps", bufs=2, space=bass.MemorySpace.PSUM) as qkv_ps,
                  tc.tile_pool(name="row_ps", bufs=1, space=bass.MemorySpace.PSUM) as row_ps,
                  tc.tile_pool(name="tr_ps", bufs=1, space=bass.MemorySpace.PSUM) as tr_ps,
                  tc.tile_pool(name="evac", bufs=2) as evac,
                  tc.tile_pool(name="rows", bufs=2) as rows,
                  tc.tile_pool(name="tmps", bufs=3) as tmps,
              ):
                  wqkv = wpool.tile([128, 3 * DIM], bf16, tag="wqkv")
                  nc.sync.dma_start(wqkv[:], wqkv_d[:])

                  for c in range(NCHUNK):
                      cs = bass.ts(c, 512)
                      ps_q = qkv_ps.tile([128, 512], f32, tag="ps_q")
                      ps_k = qkv_ps.tile([128, 512], f32, tag="ps_k")
                      ps_v = qkv_ps.tile([128, 512], f32, tag="ps_v")
                      # one 2MB DMA per chunk: [p, dt, col] <- xt[128*dt + p, 512c + col]
                      xt_t = xt_pool.tile([128, 8, 512], bf16, tag="xt")
                      nc.sync.dma_start(
                          xt_t[:],
                          xt_d[:, 512 * c : 512 * (c + 1)].rearrange(
                              "(dt p) col -> p dt col", p=128
                          ),
                      )
                      vew_c = xt_pool.tile([128, 512], f32, tag="vew_c")
                      nc.sync.dma_start(vew_c[:], vew_d[:, cs])
                      for dt in range(8):
                          st, sp = dt == 0, dt == 7
                          nc.tensor.matmul(ps_q[:], wqkv[:, bass.ts(dt, 128)], xt_t[:, dt, :], start=st, stop=sp)
                          nc.tensor.matmul(ps_k[:], wqkv[:, DIM + 128 * dt : DIM + 128 * (dt + 1)], xt_t[:, dt, :], start=st, stop=sp)
                          nc.tensor.matmul(ps_v[:], wqkv[:, 2 * DIM + 128 * dt : 2 * DIM + 128 * (dt + 1)], xt_t[:, dt, :], start=st, stop=sp)
                      if c == 0 and _rep == 0:
                          load_rope_tables()

                      # ---- v: transpose [d,t]->[t,d] per 128-tile, add ve ----
                      vTc = evac.tile([128, 512], f32, tag="vTc")
                      nc.scalar.copy(vTc[:], ps_v[:])
                      for j in range(4):
                          i = 4 * c + j
                          ps_t = tr_ps.tile([128, 128], f32, tag="ps_t")
                          nc.tensor.transpose(ps_t[:], vTc[:, bass.ts(j, 128)], ident[:])
                          nc.vector.tensor_tensor(
                              v_sb[:, bass.ts(i, 128)], ps_t[:], vew_c[:, bass.ts(j, 128)], ALU.add
                          )

                      # ---- q, k: evac -> (sq -> rowsum -> rsq -> bcast) | rope,
                      #      then in-place normalize ----
                      for which, ps_x, dstT in (("q", ps_q, qnT), ("k", ps_k, knT)):
                          dcs = dstT[:, cs]
                          nc.scalar.copy(dcs, ps_x[:])
                          sqc = tmps.tile([128, 512], f32r, tag="sqc")
                          nc.scalar.activation(sqc[:], ps_x[:], FN.Square)
                          ps_row = row_ps.tile([1, 512], f32, tag="ps_row")
                          nc.tensor.matmul(ps_row[:], onc[:], sqc[:], start=True, stop=True)
                          # rsq = exp(-0.5 * ln(mean + eps)) [* SCALE for q]
                          lnr = rows.tile([1, 512], f32, tag="lnr")
                          nc.scalar.activation(lnr[:], ps_row[:], FN.Ln, bias=EPS, scale=1.0 / HEAD_DIM)
                          rsq = rows.tile([1, 512], f32, tag="rsq")
                          nc.scalar.activation(
                              rsq[:], lnr[:], FN.Exp,
                              bias=(ln_scale_q if which == "q" else 0.0), scale=-0.5,
                          )
                          bc = tmps.tile([128, 512], f32, tag="bc")
                          nc.gpsimd.partition_broadcast(bc[:], rsq[:])
                          # rope in place on dstT: build the 64-partition-rotated
                          # copy on Pool, then full-width mult/add on DVE
                          # (smat rows are pre-shifted on the host; rows 32:64
                          # and 96:128 of smat are zero so the add is identity
                          # there)
                          dsh = tmps.tile([128, 512], f32, tag="dsh")
                          nc.gpsimd.tensor_copy(dsh[0:64, :], dcs[64:128, :].bitcast(f32))
                          nc.gpsimd.tensor_copy(dsh[64:128, :], dcs[0:64, :].bitcast(f32))
                          ut = tmps.tile([128, 512], f32, tag="ut")
                          nc.vector.tensor_tensor(ut[:], dsh[:], smat[:, cs], ALU.mult)
                          nc.vector.tensor_tensor(dcs, dcs.bitcast(f32), cmat[:, cs], ALU.mult)
                          nc.vector.tensor_tensor(dcs, dcs.bitcast(f32), ut[:], ALU.add)
                          # normalize (per-column factor, broadcast across partitions)
                          nc.vector.tensor_tensor(dcs, dcs.bitcast(f32), bc[:], ALU.mult)

              PHASE_MARKS["p3"] = nc.next_id()
              # ================= Phase 3: causal attention ======================
              with (
                  tc.tile_pool(name="s_ps", bufs=4, space=bass.MemorySpace.PSUM) as s_ps,
                  tc.tile_pool(name="y_ps", bufs=2, space=bass.MemorySpace.PSUM) as y_ps,
                  tc.tile_pool(name="r_ps", bufs=2, space=bass.MemorySpace.PSUM) as r_ps,
                  tc.tile_pool(name="pt", bufs=6) as pt_pool,
                  tc.tile_pool(name="att_sb", bufs=2) as att_sb,
              ):
                  for c in range(NCHUNK):
                      cs = bass.ts(c, 512)
                      n_s = 4 * (c + 1)
                      ps_y = y_ps.tile([128, 512], f32, tag="ps_y")
                      ps_r = r_ps.tile([1, 512], f32, tag="ps_r")
                      pacc = att_sb.tile([128, 512], f32r, tag="pacc")
                      # diagonal tiles first (their mask lengthens the chain);
                      # the stop tile is then mask-free for a short chunk tail
                      order = list(range(4 * c, n_s)) + list(range(0, 4 * c))
                      for pos, i in enumerate(order):
                          st, sp = pos == 0, pos == n_s - 1
                          ps_S = s_ps.tile([128, 512], f32, tag="ps_S")
                          nc.tensor.matmul(
                              ps_S[:], knT[:, bass.ts(i, 128)], qnT[:, cs],
                              start=True, stop=True,
                          )
                          k_idx = i - 4 * c
                          if k_idx >= 0:
                              w = 128 * (k_idx + 1)
                              nc.vector.tensor_tensor(
                                  ps_S[:, 0:w],
                                  ps_S[:, 0:w],
                                  maskc[:, 512 * k_idx : 512 * k_idx + w],
                                  ALU.add,
                              )
                          pT = pt_pool.tile([128, 512], f32r, tag="pT")
                          nc.scalar.activation(pT[:], ps_S[:], FN.Exp)
                          nc.tensor.matmul(
                              ps_y[:], v_sb[:, bass.ts(i, 128)], pT[:],
                              start=st, stop=sp,
                          )
                          # softmax denominator: even positions via the PE
                          # ones-matmul chain, odd positions summed on DVE and
                          # folded in with one cleanup matmul per chunk
                          if pos % 2 == 0:
                              nc.tensor.matmul(
                                  ps_r[:], onc[:], pT[:],
                                  start=st, stop=False,
                              )
                          elif pos == 1:
                              nc.vector.tensor_copy(pacc[:], pT[:].bitcast(f32))
                          else:
                              nc.vector.tensor_tensor(
                                  pacc[:], pacc[:].bitcast(f32), pT[:].bitcast(f32),
                                  ALU.add,
                              )
                      nc.tensor.matmul(ps_r[:], onc[:], pacc[:], start=False, stop=True)
                      # normalize y chunk by 1/rowsum and ship to a2a buffer
                      rrec = att_sb.tile([1, 512], f32, tag="rrec")
                      nc.vector.reciprocal(rrec[:], ps_r[:])
                      brs = att_sb.tile([128, 512], f32, tag="brs")
                      nc.gpsimd.partition_broadcast(brs[:], rrec[:])
                      yn = att_sb.tile([128, 512], bf16, tag="yn")
                      nc.vector.tensor_tensor(yn[:], ps_y[:], brs[:], ALU.mult)
                      nc.scalar.dma_start(a2a_in[128 * c : 128 * (c + 1), :], yn[:])

              PHASE_MARKS["cc"] = nc.next_id()
              nc.gpsimd.collective_compute(
                  "AllToAll",
                  ALU.bypass,
                  replica_groups=[list(range(N_CORES))],
                  ins=[a2a_in[:].opt()],
                  outs=[a2a_out[:].opt()],
              )

              PHASE_MARKS["p4"] = nc.next_id()
              # ================= Phase 4: output projection =====================
              # accumulate over heads into 8 PSUM banks; stream per-head inputs
              with (
                  tc.tile_pool(name="proj_in", bufs=3) as proj_in,
                  tc.tile_pool(name="o_ps", bufs=1, space=bass.MemorySpace.PSUM) as o_ps,
                  tc.tile_pool(name="outp", bufs=3) as outp,
              ):
                  ps_os = []
                  for j in range(8):
                      ps_os.append(
                          o_ps.tile([128, 512], f32, tag=f"ps_o{j}", name=f"ps_o{j}")
                      )
                  for hh in range(8):
                      yT_h = proj_in.tile([128, TSLICE], bf16, tag="yT_h")
                      nc.sync.dma_start(yT_h[:], a2a_out[128 * hh : 128 * (hh + 1), :])
                      pw_h = proj_in.tile([128, DIM], bf16, tag="pw_h")
                      nc.scalar.dma_start(pw_h[:], pw_d[:, bass.ts(hh, DIM)])
                      for m in range(4):
                          for dc in range(2):
                              nc.tensor.matmul(
                                  ps_os[2 * m + dc][:],
                                  yT_h[:, bass.ts(m, 128)],
                                  pw_h[:, bass.ts(dc, 512)],
                                  start=(hh == 0), stop=(hh == 7),
                              )
                  for m in range(4):
                      for dc in range(2):
                          ob = outp.tile([128, 512], f32, tag="ob")
                          if dc == 0:
                              nc.vector.tensor_copy(ob[:], ps_os[2 * m + dc][:])
                              nc.scalar.dma_start(
                                  out_d[128 * m : 128 * (m + 1), 0:512], ob[:]
                              )
                          else:
                              nc.scalar.copy(ob[:], ps_os[2 * m + dc][:])
                              nc.sync.dma_start(
                                  out_d[128 * m : 128 * (m + 1), 512:1024], ob[:]
                              )

    nc.finalize()
    _force_single_act_table(nc)
    return nc


_PROGRAM = None


def _get_program():
    global _PROGRAM
    if _PROGRAM is None:
        _PROGRAM = _build_program()
    return _PROGRAM


def _host_prep(x, ve, qkv_w, lambdas, proj_w):
    x = np.asarray(x, dtype=np.float32).reshape(T, DIM)
    ve = np.asarray(ve, dtype=np.float32).reshape(T, HDIM)
    qkv_w = np.asarray(qkv_w, dtype=np.float32)
    lam = np.asarray(lambdas, dtype=np.float32)
    proj_w = np.asarray(proj_w, dtype=np.float32)

    import ml_dtypes
    xt = np.ascontiguousarray(x.T).astype(ml_dtypes.bfloat16)    # [DIM, T]

    # rope tables
    nfreq = HEAD_DIM // 4
    ang = (1.0 / 1024.0) ** np.linspace(0.0, 1.0, nfreq, dtype=np.float32)
    theta = np.arange(T, dtype=np.float32)[:, None] * ang[None, :]     # [T, 32]
    cosT = np.cos(theta).T.astype(np.float32)              # [32, T]
    sinT = np.sin(theta).T.astype(np.float32)
    cos1 = np.concatenate([cosT, np.ones((32, T), np.float32)], axis=0)   # [64, T]
    sin0 = np.concatenate([sinT, np.zeros((32, T), np.float32)], axis=0)  # [64, T]

    # causal masks for the 4 diagonal sub-positions
    maskc = np.zeros((128, 2048), np.float32)
    tri = np.where(
        np.arange(128)[:, None] > np.arange(128)[None, :], MASK_NEG, 0.0
    ).astype(np.float32)
    for k in range(4):
        maskc[:, 512 * k : 512 * k + 128 * k] = MASK_NEG
        maskc[:, 512 * k + 128 * k : 512 * k + 128 * (k + 1)] = tri

    ones_col = _trunc22(np.ones((128, 1), np.float32))
    ident = np.eye(128, dtype=np.float32)

    in_maps = []
    for h in range(N_CORES):
        hs = slice(128 * h, 128 * (h + 1))
        # weight layout: w[p, dt*128 + m] = W[m, dt*128 + p]
        Wq = qkv_w[0, hs, :]                                # [128, DIM]
        Wk = qkv_w[1, hs, :]
        Wv = qkv_w[2, hs, :] * lam[0]
        def wlay(W):
            # [m, (dt p)] -> [p, (dt m)]
            a = W.reshape(128, 8, 128)                      # [m, dt, p]
            return _trunc22(np.ascontiguousarray(a.transpose(2, 1, 0).reshape(128, DIM)))
        # vew[p, i*128 + c] = lam1 * ve[i*128 + p, h*128 + c]
        veh = (ve[:, hs] * lam[1]).reshape(NTT, 128, 128)   # [i, p, c]
        vew = np.ascontiguousarray(veh.transpose(1, 0, 2).reshape(128, T))
        # pw[p, n*DIM + D] = proj_w[D, 128n + p]
        pwh = proj_w.T.reshape(8, 128, DIM)                 # [n, e_p, D]
        pw = np.ascontiguousarray(pwh.transpose(1, 0, 2).reshape(128, 8 * DIM)).astype(ml_dtypes.bfloat16)
        in_maps.append(
            {
                "xt": xt,
                "wqkv": np.concatenate([wlay(Wq), wlay(Wk), wlay(Wv)], axis=1).astype(ml_dtypes.bfloat16),
                "vew": vew.astype(np.float32),
                "cos1": cos1,
                "sin0": sin0,
                "maskc": maskc,
                "pw": pw,
                "ones_col": ones_col,
                "ident": ident,
            }
        )
    return in_maps


def kernel(x, ve, qkv_w, lambdas, proj_w):
    in_maps = _host_prep(x, ve, qkv_w, lambdas, proj_w)
    nc = _get_program()
    res = run_bass_kernel_spmd(nc, in_maps, list(range(N_CORES)))
    out = np.concatenate([res.results[c]["out"] for c in range(N_CORES)], axis=0)
    return out.reshape(B, T, DIM).astype(np.float32)


# ---------------------------------------------------------------------------
# Timing support (test.py only): run the program with device-resident inputs
# so repeated executions measure device time, and difference two repeat
# factors to cancel dispatch overhead.
# ---------------------------------------------------------------------------

def make_runner(in_maps, repeat=1):
    import jax
    from jax.sharding import Mesh, PartitionSpec, NamedSharding
    from jax.experimental.shard_map import shard_map
    from concourse import bass2jax
    from concourse.bass2jax import _bass_exec_p, partition_id_tensor

    bass2jax.install_neuronx_cc_hook()
    nc = _build_program(repeat)

    in_names, out_names, out_avals, zero_outs = [], [], [], []
    partition_name = nc.partition_id_tensor.name if nc.partition_id_tensor else None
    for alloc in nc.m.functions[0].allocations:
        if not isinstance(alloc, mybir.MemoryLocationSet):
            continue
        name = alloc.memorylocations[0].name
        if alloc.kind == "ExternalInput":
            if name != partition_name:
                in_names.append(name)
        elif alloc.kind == "ExternalOutput":
            out_names.append(name)
            shape = tuple(alloc.tensor_shape)
            dtype = mybir.dt.np(alloc.dtype)
            out_avals.append(jax.core.ShapedArray(shape, dtype))
            zero_outs.append(np.zeros(shape, dtype))
    n_params = len(in_names)
    n_outs = len(out_avals)
    all_in_names = list(in_names) + out_names
    if partition_name is not None:
        all_in_names.append(partition_name)
    donate = tuple(range(n_params, n_params + n_outs))

    def _body(*args):
        operands = list(args)
        if partition_name is not None:
            operands.append(partition_id_tensor())
        outs = _bass_exec_p.bind(
            *operands,
            out_avals=tuple(out_avals),
            in_names=tuple(all_in_names),
            out_names=tuple(out_names),
            lowering_input_output_aliases=(),
            sim_require_finite=True,
            sim_require_nnan=True,
            nc=nc,
        )
        return tuple(outs)

    devices = jax.devices()[:N_CORES]
    mesh = Mesh(np.asarray(devices), ("core",))
    in_specs = (PartitionSpec("core"),) * (n_params + n_outs)
    out_specs = (PartitionSpec("core"),) * n_outs
    fn = jax.jit(
        shard_map(_body, mesh=mesh, in_specs=in_specs, out_specs=out_specs, check_rep=False),
        donate_argnums=donate,
        keep_unused=True,
    )
    sh = NamedSharding(mesh, PartitionSpec("core"))
    concat_in = [
        jax.device_put(
            np.concatenate([np.asarray(in_maps[c][nm]) for c in range(N_CORES)], axis=0), sh
        )
        for nm in in_names
    ]
    zero_glob = [np.zeros((N_CORES * z.shape[0], *z.shape[1:]), z.dtype) for z in zero_outs]

    def run_once():
        zs = [jax.device_put(z, sh) for z in zero_glob]
        outs = fn(*concat_in, *zs)
        for o in outs:
            o.block_until_ready()
        return outs

    return run_once
